# revision 12
# baseline (speedup 1.0000x reference)
"""GCNConv-with-constraint kernel for 8 Trainium2 NeuronCores.

Strategy (v2): nodes are sharded across the 8 cores by destination. The whole
(fp16) x table fits in SBUF (50048 x 128 x 2B = 12.8MB), so per-edge source
rows are fetched with one-hot PE matmuls against on-chip 128-row table blocks.

v2 change vs v1: the one-hot gather matrix F[row, edge] (norm-scaled) is
precomputed on the HOST and streamed from DRAM, instead of being built
on-chip as F_T by the DVE and transposed through the PE (the PE-mode
transpose was ~275ns/tile, ~40% of the per-tile PE budget, and it also cost
a DVE build plus an ACT PSUM->SBUF copy). DMA streams F at ~97GB/s which the
16 SDMA engines absorb while the PE computes.

Per core:
  - dsts are split into superblocks of 512 (one PSUM bank of fp32 [128ch,512]).
  - edges of a superblock are bucketed by source block (128 table rows),
    buckets padded to a multiple of 64 so every 64-slot chunk is block-pure
    and the gather matmul writes a 64-aligned PSUM partition range (PE
    base-partition constraint: offsets 0/64 only).
  - per 128-edge tile:
      DMA  F[r, e] tile (fp16, one-hot columns scaled by the edge norm)
      DVE  sel[e, d] = (d == dstloc_e) via per-tile tensor_scalar
      PE   2 gather matmuls  msgs[64-chunk, c] = F[:, chunk]^T @ xblk[s]
           1 scatter matmul  agg[c, d] += msgs^T @ sel   (PSUM accumulate)
      Act  copies msgs PSUM->SBUF (fp16)
  - superblock epilogue: agg -> SBUF, out^T = WnT^T @ agg + b, DMA out.

The x table is per-core ROTATED by the core's node offset so self-loop edges
(dst d -> table row d_local) hit core-independent buckets; bucket sizes are
maxed over the 8 cores so the single SPMD program is valid for all cores.
Host does structure/metadata only: degree bincount, norm coefficients, edge
sort/padding, the F table, W column-renorm (128x128), fp16 casts, final
transpose/concat.
"""

import math
import os
from contextlib import ExitStack, nullcontext

import numpy as np

import concourse.bass as bass
import concourse.tile as tile
from concourse import bacc, mybir
from concourse.bass_utils import run_bass_kernel_spmd

N_CORES = 8
C = 128  # in/out channels
P = 128  # partitions / edge-tile size
SBW = 512  # dst superblock width (one fp32 PSUM bank)
CHUNK = 64  # gather sub-matmul width (PSUM base partition must be 0/32/64)
QUAD = 8  # tiles processed per group (shared DVE ops / Act copies)

f16 = mybir.dt.float16
f32 = mybir.dt.float32

# test.py introspection: the last BassKernelResults
LAST_RESULTS = None


def _prep(x, edge_index, W, b):
    """Host-side sharding/metadata prep. Returns per-core input maps and the
    common (data-dependent, core-uniform) structure baked into the program."""
    x = np.asarray(x)
    N = x.shape[0]
    assert N % N_CORES == 0, N
    npc = N // N_CORES
    NSB = math.ceil(npc / SBW)
    NBLK = math.ceil(N / P)
    Npad = NBLK * P
    nk = NSB * NBLK

    src = np.asarray(edge_index[0], dtype=np.int64)
    dst = np.asarray(edge_index[1], dtype=np.int64)

    deg = np.bincount(dst, minlength=N).astype(np.float64) + 1.0
    dinv = 1.0 / np.sqrt(deg)
    norm = (dinv[src] * dinv[dst]).astype(np.float32)

    core = dst // npc
    dstl = dst - core * npc
    r = (src - core * npc) % N  # rotated table row of the source

    key = (dstl >> 9) * NBLK + (r >> 7)

    # self loops: dst d -> rotated row d_local; identical structure on all cores
    arN = np.arange(npc, dtype=np.int64)
    self_key = (arN >> 9) * NBLK + (arN >> 7)
    cnt_self = np.bincount(self_key, minlength=nk)

    cnt = np.zeros((N_CORES, nk), dtype=np.int64)
    for c in range(N_CORES):
        cnt[c] = np.bincount(key[core == c], minlength=nk) + cnt_self
    m = ((cnt.max(axis=0) + CHUNK - 1) // CHUNK) * CHUNK  # padded bucket sizes

    # per-SB chunk lists (+pad chunks so each SB is a whole number of QUADs)
    tiles_sb = []
    tile_off = []
    chunk_block = []  # per SB: list of source-block ids, one per CHUNK-slot chunk
    bucket_off = np.zeros(nk, dtype=np.int64)  # global slot offset per bucket
    toff = 0
    for sb in range(NSB):
        msb = m[sb * NBLK : (sb + 1) * NBLK]
        offs = np.concatenate([[0], np.cumsum(msb)[:-1]])
        bucket_off[sb * NBLK : (sb + 1) * NBLK] = toff * P + offs
        slots = int(msb.sum())
        slots_pad = math.ceil(slots / (P * QUAD)) * (P * QUAD)
        blocks = []
        for s in range(NBLK):
            blocks.extend([s] * (int(msb[s]) // CHUNK))
        blocks.extend([0] * ((slots_pad - slots) // CHUNK))
        chunk_block.append(blocks)
        tile_off.append(toff)
        tiles_sb.append(slots_pad // P)
        toff += slots_pad // P
    n_tiles = toff
    n_slots = n_tiles * P

    # host W renorm: Wn = W * min(1, 1/||W[:,i]||); ship WnT = Wn^T [in, out]
    Wf = np.asarray(W, dtype=np.float64)
    norms = np.sqrt((Wf**2).sum(axis=0, keepdims=True))
    scale = np.where(norms > 1.0, 1.0 / norms, 1.0)
    WnT = np.ascontiguousarray((np.asarray(W, np.float32) * scale.astype(np.float32)).T)
    bvec = np.ascontiguousarray(np.asarray(b, dtype=np.float32).reshape(C, 1))

    iotaS = np.ascontiguousarray(
        np.broadcast_to(np.arange(SBW, dtype=np.float16)[None, :], (P, SBW))
    )

    in_maps = []
    for c in range(N_CORES):
        mask = core == c
        lo = c * npc
        allr = np.concatenate([r[mask], arN])
        alldl = np.concatenate([dstl[mask], arN])
        allnm = np.concatenate([norm[mask], (dinv[lo : lo + npc] ** 2).astype(np.float32)])
        allkey = np.concatenate([key[mask], self_key])

        order = np.argsort(allkey, kind="stable")
        cntc = np.bincount(allkey, minlength=nk)
        starts = np.concatenate([[0], np.cumsum(cntc)[:-1]])
        rank = np.arange(len(allkey)) - np.repeat(starts, cntc)
        slot = bucket_off[allkey[order]] + rank

        # host-built gather one-hot: ftab[r, slot] = norm_slot at r = srcloc
        ftab = np.zeros((P, n_slots), np.float16)
        ftab[(allr[order] & (P - 1)).astype(np.int64), slot] = allnm[order].astype(
            np.float16
        )
        D = np.full(n_slots, -1.0, np.float32)
        D[slot] = (alldl[order] & (SBW - 1)).astype(np.float32)

        xrot = np.concatenate([x[lo:], x[:lo]]).astype(np.float16)
        if Npad > N:
            xrot = np.concatenate([xrot, np.zeros((Npad - N, C), np.float16)])
        xpm = np.ascontiguousarray(
            xrot.reshape(NBLK, P, C).transpose(1, 0, 2).reshape(P, NBLK * C)
        )

        in_maps.append(
            {
                "xtab": xpm,
                "ftab": np.ascontiguousarray(ftab),
                "dsts": np.ascontiguousarray(
                    D.reshape(n_tiles, P).T
                ),
                "wnT": WnT,
                "bvec": bvec,
                "iotaS": iotaS,
            }
        )

    structure = dict(
        N=N,
        npc=npc,
        NSB=NSB,
        NBLK=NBLK,
        n_tiles=n_tiles,
        tiles_sb=tiles_sb,
        tile_off=tile_off,
        chunk_block=chunk_block,
    )
    return in_maps, structure


def _build_program(st, repeat=1, ablate=None):
    N, NSB, NBLK, n_tiles = st["N"], st["NSB"], st["NBLK"], st["n_tiles"]
    nc = bacc.Bacc("TRN2", target_bir_lowering=False, debug=False, num_devices=N_CORES)

    xtab = nc.dram_tensor("xtab", [P, NBLK * C], f16, kind="ExternalInput").ap()
    ftab = nc.dram_tensor("ftab", [P, n_tiles * P], f16, kind="ExternalInput").ap()
    dsts = nc.dram_tensor("dsts", [P, n_tiles], f32, kind="ExternalInput").ap()
    wnT = nc.dram_tensor("wnT", [C, C], f32, kind="ExternalInput").ap()
    bvec = nc.dram_tensor("bvec", [C, 1], f32, kind="ExternalInput").ap()
    iotaS = nc.dram_tensor("iotaS", [P, SBW], f16, kind="ExternalInput").ap()
    outt = nc.dram_tensor("outt", [C, NSB * SBW], f32, kind="ExternalOutput").ap()

    Copy = mybir.ActivationFunctionType.Copy
    Op = mybir.AluOpType

    with tile.TileContext(nc) as tc, ExitStack() as ctx:
        cpool = ctx.enter_context(tc.tile_pool(name="const", bufs=1))
        xsb = cpool.tile([P, NBLK, C], f16, tag="xsb")
        nc.sync.dma_start(xsb[:], xtab[:])
        iotaS_sb = cpool.tile([P, SBW], f16, tag="iotaS")
        nc.sync.dma_start(iotaS_sb[:], iotaS[:])
        wnT_sb = cpool.tile([C, C], f32, tag="wnT")
        nc.sync.dma_start(wnT_sb[:], wnT[:])
        bias_sb = cpool.tile([C, 1], f32, tag="bias")
        nc.sync.dma_start(bias_sb[:], bvec[:])

        # edge metadata (dst one-hot scalars) is SBUF-resident, loaded once
        dl_all = cpool.tile([P, n_tiles], f32, tag="dl_all")
        nc.sync.dma_start(dl_all[:], dsts[:])

        fpool = ctx.enter_context(tc.tile_pool(name="f", bufs=4))
        selpool = ctx.enter_context(tc.tile_pool(name="sel", bufs=3))
        msbpool = ctx.enter_context(tc.tile_pool(name="msb", bufs=4))
        asbpool = ctx.enter_context(tc.tile_pool(name="aggsb", bufs=2))
        mpsp = ctx.enter_context(tc.tile_pool(name="mps", bufs=2, space="PSUM"))
        aggp = ctx.enter_context(tc.tile_pool(name="aggps", bufs=2, space="PSUM"))
        outp = ctx.enter_context(tc.tile_pool(name="outps", bufs=1, space="PSUM"))

        out_stage = cpool.tile([C, NSB, SBW], f32, tag="out_stage")

        sel_const = None
        if ablate == "sel":
            sel_const = cpool.tile([P, QUAD, SBW], f16, tag="sel_const")
            for t in range(QUAD):
                nc.vector.tensor_scalar(
                    out=sel_const[:, t, :], in0=iotaS_sb[:],
                    scalar1=bias_sb[:P, :], scalar2=None, op0=Op.is_equal)

        aggs = {}

        def emit_front(sb, q):
            t0 = st["tile_off"][sb]
            nt = st["tiles_sb"][sb]
            blocks = st["chunk_block"][sb]
            dl = dl_all[:, t0 : t0 + nt]
            if q == 0:
                aggs[sb] = aggp.tile([C, SBW], f32, tag="agg", name=f"agg_sb{sb}")
            fq = fpool.tile([P, QUAD, P], f16, tag="fq")
            nc.sync.dma_start(
                fq[:], ftab[:, (t0 + QUAD * q) * P : (t0 + QUAD * (q + 1)) * P]
            )
            if ablate == "sel":
                sel = sel_const
            else:
                sel = selpool.tile([P, QUAD, SBW], f16, tag="sel")
                for t in range(QUAD):
                    k = QUAD * q + t
                    nc.vector.tensor_scalar(
                        out=sel[:, t, :], in0=iotaS_sb[:],
                        scalar1=dl[:, k : k + 1], scalar2=None,
                        op0=Op.is_equal)
            msb = msbpool.tile([P, QUAD, C], f16, tag="msb")
            if ablate == "gather":
                nc.scalar.activation(msb[:], fq[:], Copy)
            else:
                mps = mpsp.tile([P, QUAD, C], f32, tag="mps")
                for t in range(QUAD):
                    for g in range(P // CHUNK):
                        s = blocks[(QUAD * q + t) * (P // CHUNK) + g]
                        nc.tensor.matmul(
                            mps[CHUNK * g : CHUNK * (g + 1), t, :],
                            lhsT=fq[:, t, CHUNK * g : CHUNK * (g + 1)],
                            rhs=xsb[:, s, :], start=True, stop=True)
                nc.scalar.activation(msb[:], mps[:], Copy)
            return (sb, q, msb, sel)

        def emit_back(rec):
            sb, q, msb, sel = rec
            nq = st["tiles_sb"][sb] // QUAD
            agg = aggs[sb]
            for t in range(QUAD):
                if ablate == "scatter" and not (q == 0 and t < 2):
                    continue
                nc.tensor.matmul(
                    agg[:], lhsT=msb[:, t, :], rhs=sel[:, t, :],
                    start=(q == 0 and t == 0),
                    stop=(q == 0 and t == 1)
                    if ablate == "scatter"
                    else (q == nq - 1 and t == QUAD - 1))
            if q == nq - 1:
                agg_sb = asbpool.tile([C, SBW], f32, tag="aggsb")
                nc.scalar.activation(agg_sb[:], agg[:], Copy)
                outT_ps = outp.tile([C, SBW], f32, tag="outps")
                nc.tensor.matmul(
                    outT_ps[:], lhsT=wnT_sb[:], rhs=agg_sb[:], start=True, stop=True
                )
                nc.vector.tensor_scalar(
                    out=out_stage[:, sb, :],
                    in0=outT_ps[:],
                    scalar1=bias_sb[:],
                    scalar2=None,
                    op0=Op.add,
                )

        DELAY = 1  # quads of scatter-deferral: hides the ACT msb copy under
        # the next quads' gather matmuls so the PE never waits on ACT
        loop = tc.For_i(0, repeat) if repeat > 1 else nullcontext()
        with loop:
            pending = []
            for sb in range(NSB):
                for q in range(st["tiles_sb"][sb] // QUAD):
                    pending.append(emit_front(sb, q))
                    if len(pending) > DELAY:
                        emit_back(pending.pop(0))
            for rec in pending:
                emit_back(rec)
            # one fat output DMA per execution (128 descriptors of 26KB)
            nc.sync.dma_start(outt[:], out_stage[:])

    nc.compile()
    return nc


def kernel(x, edge_index, W, b):
    global LAST_RESULTS
    x = np.asarray(x)
    N = x.shape[0]
    assert x.shape[1] == C and W.shape == (C, C)

    in_maps, st = _prep(x, edge_index, W, b)
    nc = _build_program(st)

    os.environ.setdefault("BASS_NEVER_TRACE", "1")  # no NTFF hook in this env
    res = run_bass_kernel_spmd(nc, in_maps, list(range(N_CORES)))
    LAST_RESULTS = res

    npc = st["npc"]
    shards = []
    for s in range(N_CORES):
        lo = s * npc
        hi = min((s + 1) * npc, N)
        outT = res.results[s]["outt"]  # [C, NSB*SBW]
        shards.append(outT[:, : hi - lo].T)
    return np.ascontiguousarray(np.concatenate(shards, axis=0), dtype=np.float32)


# revision 17
# speedup vs baseline: 1.0388x; 1.0388x over previous
"""GCNConv-with-constraint kernel for 8 Trainium2 NeuronCores.

Strategy (v2): nodes are sharded across the 8 cores by destination. The whole
(fp16) x table fits in SBUF (50048 x 128 x 2B = 12.8MB), so per-edge source
rows are fetched with one-hot PE matmuls against on-chip 128-row table blocks.

v2 change vs v1: the one-hot gather matrix F[row, edge] (norm-scaled) is
precomputed on the HOST and streamed from DRAM, instead of being built
on-chip as F_T by the DVE and transposed through the PE (the PE-mode
transpose was ~275ns/tile, ~40% of the per-tile PE budget, and it also cost
a DVE build plus an ACT PSUM->SBUF copy). DMA streams F at ~97GB/s which the
16 SDMA engines absorb while the PE computes.

Per core:
  - dsts are split into superblocks of 512 (one PSUM bank of fp32 [128ch,512]).
  - edges of a superblock are bucketed by source block (128 table rows),
    buckets padded to a multiple of 64 so every 64-slot chunk is block-pure
    and the gather matmul writes a 64-aligned PSUM partition range (PE
    base-partition constraint: offsets 0/64 only).
  - per 128-edge tile:
      DMA  F[r, e] tile (fp16, one-hot columns scaled by the edge norm)
      DVE  sel[e, d] = (d == dstloc_e) via per-tile tensor_scalar
      PE   2 gather matmuls  msgs[64-chunk, c] = F[:, chunk]^T @ xblk[s]
           1 scatter matmul  agg[c, d] += msgs^T @ sel   (PSUM accumulate)
      Act  copies msgs PSUM->SBUF (fp16)
  - superblock epilogue: agg -> SBUF, out^T = WnT^T @ agg + b, DMA out.

The x table is per-core ROTATED by the core's node offset so self-loop edges
(dst d -> table row d_local) hit core-independent buckets; bucket sizes are
maxed over the 8 cores so the single SPMD program is valid for all cores.
Host does structure/metadata only: degree bincount, norm coefficients, edge
sort/padding, the F table, W column-renorm (128x128), fp16 casts, final
transpose/concat.
"""

import math
import os
from contextlib import ExitStack, nullcontext

import numpy as np

import concourse.bass as bass
import concourse.tile as tile
from concourse import bacc, mybir
from concourse.bass_utils import run_bass_kernel_spmd

N_CORES = 8
C = 128  # in/out channels
P = 128  # partitions / edge-tile size
SBW = 512  # dst superblock width (one fp32 PSUM bank)
CHUNK = 64  # gather sub-matmul width (PSUM base partition must be 0/32/64)
QUAD = 8  # tiles processed per group (shared DVE ops / Act copies)

f16 = mybir.dt.float16
f32 = mybir.dt.float32

# test.py introspection: the last BassKernelResults
LAST_RESULTS = None


def _prep(x, edge_index, W, b):
    """Host-side sharding/metadata prep. Returns per-core input maps and the
    common (data-dependent, core-uniform) structure baked into the program."""
    x = np.asarray(x)
    N = x.shape[0]
    assert N % N_CORES == 0, N
    npc = N // N_CORES
    NSB = math.ceil(npc / SBW)
    NBLK = math.ceil(N / P)
    Npad = NBLK * P
    nk = NSB * NBLK

    src = np.asarray(edge_index[0], dtype=np.int64)
    dst = np.asarray(edge_index[1], dtype=np.int64)

    deg = np.bincount(dst, minlength=N).astype(np.float64) + 1.0
    dinv = 1.0 / np.sqrt(deg)
    norm = (dinv[src] * dinv[dst]).astype(np.float32)

    core = dst // npc
    dstl = dst - core * npc
    r = (src - core * npc) % N  # rotated table row of the source

    key = (dstl >> 9) * NBLK + (r >> 7)

    # self loops: dst d -> rotated row d_local; identical structure on all cores
    arN = np.arange(npc, dtype=np.int64)
    self_key = (arN >> 9) * NBLK + (arN >> 7)
    cnt_self = np.bincount(self_key, minlength=nk)

    cnt = np.zeros((N_CORES, nk), dtype=np.int64)
    for c in range(N_CORES):
        cnt[c] = np.bincount(key[core == c], minlength=nk) + cnt_self
    m = ((cnt.max(axis=0) + CHUNK - 1) // CHUNK) * CHUNK  # padded bucket sizes

    # per-SB chunk lists (+pad chunks so each SB is a whole number of QUADs)
    tiles_sb = []
    real_tiles_sb = []
    tile_off = []
    chunk_block = []  # per SB: list of source-block ids, one per CHUNK-slot chunk
    bucket_off = np.zeros(nk, dtype=np.int64)  # global slot offset per bucket
    toff = 0
    for sb in range(NSB):
        msb = m[sb * NBLK : (sb + 1) * NBLK]
        offs = np.concatenate([[0], np.cumsum(msb)[:-1]])
        bucket_off[sb * NBLK : (sb + 1) * NBLK] = toff * P + offs
        slots = int(msb.sum())
        slots_pad = math.ceil(slots / (P * QUAD)) * (P * QUAD)
        blocks = []
        for s in range(NBLK):
            blocks.extend([s] * (int(msb[s]) // CHUNK))
        blocks.extend([0] * ((slots_pad - slots) // CHUNK))
        chunk_block.append(blocks)
        tile_off.append(toff)
        tiles_sb.append(slots_pad // P)
        real_tiles_sb.append(math.ceil(slots / P))
        toff += slots_pad // P
    n_tiles = toff
    n_slots = n_tiles * P

    # host W renorm: Wn = W * min(1, 1/||W[:,i]||); ship WnT = Wn^T [in, out]
    Wf = np.asarray(W, dtype=np.float64)
    norms = np.sqrt((Wf**2).sum(axis=0, keepdims=True))
    scale = np.where(norms > 1.0, 1.0 / norms, 1.0)
    WnT = np.ascontiguousarray((np.asarray(W, np.float32) * scale.astype(np.float32)).T)
    bvec = np.ascontiguousarray(np.asarray(b, dtype=np.float32).reshape(C, 1))

    iotaS = np.ascontiguousarray(
        np.broadcast_to(np.arange(SBW, dtype=np.float16)[None, :], (P, SBW))
    )

    in_maps = []
    win_lo, win_hi = [], []
    for c in range(N_CORES):
        mask = core == c
        lo = c * npc
        allr = np.concatenate([r[mask], arN])
        alldl = np.concatenate([dstl[mask], arN])
        allnm = np.concatenate([norm[mask], (dinv[lo : lo + npc] ** 2).astype(np.float32)])
        allkey = np.concatenate([key[mask], self_key])

        order = np.argsort(allkey, kind="stable")
        cntc = np.bincount(allkey, minlength=nk)
        starts = np.concatenate([[0], np.cumsum(cntc)[:-1]])
        rank = np.arange(len(allkey)) - np.repeat(starts, cntc)
        slot = bucket_off[allkey[order]] + rank

        # host-built gather one-hot: ftab[r, slot] = norm_slot at r = srcloc
        ftab = np.zeros((P, n_slots), np.float16)
        ftab[(allr[order] & (P - 1)).astype(np.int64), slot] = allnm[order].astype(
            np.float16
        )
        D = np.full(n_slots, -1.0, np.float32)
        D[slot] = (alldl[order] & (SBW - 1)).astype(np.float32)
        Dm = D.reshape(n_tiles, P)
        real = Dm >= 0
        lo_t = np.where(real.any(1), np.where(real, Dm, SBW).min(1), 0)
        hi_t = np.where(real.any(1), np.where(real, Dm, -1).max(1) + 1, 2)
        lo_t = (lo_t.astype(np.int64) // 2) * 2
        hi_t = np.minimum(((hi_t.astype(np.int64) + 1) // 2) * 2, SBW)
        win_lo.append(lo_t)
        win_hi.append(hi_t)

        xrot = np.concatenate([x[lo:], x[:lo]]).astype(np.float16)
        if Npad > N:
            xrot = np.concatenate([xrot, np.zeros((Npad - N, C), np.float16)])
        xpm = np.ascontiguousarray(
            xrot.reshape(NBLK, P, C).transpose(1, 0, 2).reshape(P, NBLK * C)
        )

        in_maps.append(
            {
                "xtab": xpm,
                "ftab": np.ascontiguousarray(ftab),
                "dsts": np.ascontiguousarray(
                    D.reshape(n_tiles, P).T
                ),
                "wnT": WnT,
                "bvec": bvec,
                "iotaS": iotaS,
            }
        )

    tw_lo = np.stack(win_lo).min(axis=0)
    tw_hi = np.stack(win_hi).max(axis=0)
    structure = dict(
        tile_win=list(zip(tw_lo.tolist(), tw_hi.tolist())),
        N=N,
        npc=npc,
        NSB=NSB,
        NBLK=NBLK,
        n_tiles=n_tiles,
        tiles_sb=tiles_sb,
        real_tiles_sb=real_tiles_sb,
        tile_off=tile_off,
        chunk_block=chunk_block,
    )
    return in_maps, structure


def _build_program(st, repeat=1, ablate=None):
    N, NSB, NBLK, n_tiles = st["N"], st["NSB"], st["NBLK"], st["n_tiles"]
    nc = bacc.Bacc("TRN2", target_bir_lowering=False, debug=False, num_devices=N_CORES)

    xtab = nc.dram_tensor("xtab", [P, NBLK * C], f16, kind="ExternalInput").ap()
    ftab = nc.dram_tensor("ftab", [P, n_tiles * P], f16, kind="ExternalInput").ap()
    dsts = nc.dram_tensor("dsts", [P, n_tiles], f32, kind="ExternalInput").ap()
    wnT = nc.dram_tensor("wnT", [C, C], f32, kind="ExternalInput").ap()
    bvec = nc.dram_tensor("bvec", [C, 1], f32, kind="ExternalInput").ap()
    iotaS = nc.dram_tensor("iotaS", [P, SBW], f16, kind="ExternalInput").ap()
    outt = nc.dram_tensor("outt", [C, NSB * SBW], f32, kind="ExternalOutput").ap()

    Copy = mybir.ActivationFunctionType.Copy
    Op = mybir.AluOpType

    with tile.TileContext(nc) as tc, ExitStack() as ctx:
        cpool = ctx.enter_context(tc.tile_pool(name="const", bufs=1))
        xsb = cpool.tile([P, NBLK, C], f16, tag="xsb")
        nc.sync.dma_start(xsb[:], xtab[:])
        iotaS_sb = cpool.tile([P, SBW], f16, tag="iotaS")
        nc.sync.dma_start(iotaS_sb[:], iotaS[:])
        wnT_sb = cpool.tile([C, C], f32, tag="wnT")
        nc.sync.dma_start(wnT_sb[:], wnT[:])
        bias_sb = cpool.tile([C, 1], f32, tag="bias")
        nc.sync.dma_start(bias_sb[:], bvec[:])

        # edge metadata (dst one-hot scalars) is SBUF-resident, loaded once
        dl_all = cpool.tile([P, n_tiles], f32, tag="dl_all")
        nc.sync.dma_start(dl_all[:], dsts[:])

        fpool = ctx.enter_context(tc.tile_pool(name="f", bufs=4))
        selpool = ctx.enter_context(tc.tile_pool(name="sel", bufs=3))
        msbpool = ctx.enter_context(tc.tile_pool(name="msb", bufs=4))
        asbpool = ctx.enter_context(tc.tile_pool(name="aggsb", bufs=2))
        mpsp = ctx.enter_context(tc.tile_pool(name="mps", bufs=2, space="PSUM"))
        aggp = ctx.enter_context(tc.tile_pool(name="aggps", bufs=2, space="PSUM"))
        outp = ctx.enter_context(tc.tile_pool(name="outps", bufs=1, space="PSUM"))

        out_stage = cpool.tile([C, NSB, SBW], f32, tag="out_stage")

        sel_const = None
        if ablate == "sel":
            sel_const = cpool.tile([P, QUAD, SBW], f16, tag="sel_const")
            for t in range(QUAD):
                nc.vector.tensor_scalar(
                    out=sel_const[:, t, :], in0=iotaS_sb[:],
                    scalar1=bias_sb[:P, :], scalar2=None, op0=Op.is_equal)

        aggs = {}

        def emit_front(sb, q):
            t0 = st["tile_off"][sb]
            nt = st["tiles_sb"][sb]
            blocks = st["chunk_block"][sb]
            dl = dl_all[:, t0 : t0 + nt]
            if q == 0:
                aggs[sb] = aggp.tile([C, SBW], f32, tag="agg", name=f"agg_sb{sb}")
            nreal = st["real_tiles_sb"][sb]
            if ablate is None and QUAD * q >= nreal:
                return (sb, q, None, None)
            fq = fpool.tile([P, QUAD, P], f16, tag="fq")
            nc.sync.dma_start(
                fq[:], ftab[:, (t0 + QUAD * q) * P : (t0 + QUAD * (q + 1)) * P]
            )
            if ablate == "sel":
                sel = sel_const
            else:
                sel = selpool.tile([P, QUAD, SBW], f16, tag="sel")
                for t in range(QUAD):
                    k = QUAD * q + t
                    if k >= nreal:
                        continue
                    nc.vector.tensor_scalar(
                        out=sel[:, t, :], in0=iotaS_sb[:],
                        scalar1=dl[:, k : k + 1], scalar2=None,
                        op0=Op.is_equal)
            msb = msbpool.tile([P, QUAD, C], f16, tag="msb")
            if ablate == "gather":
                nc.scalar.activation(msb[:], fq[:], Copy)
            else:
                mps = mpsp.tile([P, QUAD, C], f32, tag="mps")
                for t in range(QUAD):
                    if QUAD * q + t >= nreal:
                        continue
                    for g in range(P // CHUNK):
                        s = blocks[(QUAD * q + t) * (P // CHUNK) + g]
                        nc.tensor.matmul(
                            mps[CHUNK * g : CHUNK * (g + 1), t, :],
                            lhsT=fq[:, t, CHUNK * g : CHUNK * (g + 1)],
                            rhs=xsb[:, s, :], start=True, stop=True)
                tq = min(QUAD, nreal - QUAD * q) if ablate is None else QUAD
                nc.scalar.activation(msb[:, :tq, :], mps[:, :tq, :], Copy)
            return (sb, q, msb, sel)

        def emit_back(rec):
            sb, q, msb, sel = rec
            nq = st["tiles_sb"][sb] // QUAD
            nreal = st["real_tiles_sb"][sb]
            agg = aggs[sb]
            if msb is not None:
                for t in range(QUAD):
                    if ablate == "scatter" and not (q == 0 and t < 2):
                        continue
                    if ablate is None and QUAD * q + t >= nreal:
                        continue
                    k = QUAD * q + t
                    first = q == 0 and t == 0
                    lo, hi = (0, SBW) if first or ablate else st["tile_win"][
                        st["tile_off"][sb] + k
                    ]
                    nc.tensor.matmul(
                        agg[:, lo:hi], lhsT=msb[:, t, :],
                        rhs=sel[:, t, lo:hi],
                        start=first,
                        stop=(q == 0 and t == 1)
                        if ablate == "scatter"
                        else (k == nreal - 1))
            if q == nq - 1:
                agg_sb = asbpool.tile([C, SBW], f32, tag="aggsb")
                nc.scalar.activation(agg_sb[:], agg[:], Copy)
                outT_ps = outp.tile([C, SBW], f32, tag="outps")
                nc.tensor.matmul(
                    outT_ps[:], lhsT=wnT_sb[:], rhs=agg_sb[:], start=True, stop=True
                )
                nc.vector.tensor_scalar(
                    out=out_stage[:, sb, :],
                    in0=outT_ps[:],
                    scalar1=bias_sb[:],
                    scalar2=None,
                    op0=Op.add,
                )

        DELAY = 1  # quads of scatter-deferral: hides the ACT msb copy under
        # the next quads' gather matmuls so the PE never waits on ACT
        loop = tc.For_i(0, repeat) if repeat > 1 else nullcontext()
        with loop:
            pending = []
            for sb in range(NSB):
                for q in range(st["tiles_sb"][sb] // QUAD):
                    pending.append(emit_front(sb, q))
                    if len(pending) > DELAY:
                        emit_back(pending.pop(0))
            for rec in pending:
                emit_back(rec)
            # one fat output DMA per execution (128 descriptors of 26KB)
            nc.sync.dma_start(outt[:], out_stage[:])

    nc.compile()
    return nc


def kernel(x, edge_index, W, b):
    global LAST_RESULTS
    x = np.asarray(x)
    N = x.shape[0]
    assert x.shape[1] == C and W.shape == (C, C)

    in_maps, st = _prep(x, edge_index, W, b)
    nc = _build_program(st)

    os.environ.setdefault("BASS_NEVER_TRACE", "1")  # no NTFF hook in this env
    res = run_bass_kernel_spmd(nc, in_maps, list(range(N_CORES)))
    LAST_RESULTS = res

    npc = st["npc"]
    shards = []
    for s in range(N_CORES):
        lo = s * npc
        hi = min((s + 1) * npc, N)
        outT = res.results[s]["outt"]  # [C, NSB*SBW]
        shards.append(outT[:, : hi - lo].T)
    return np.ascontiguousarray(np.concatenate(shards, axis=0), dtype=np.float32)
